# revision 9
# baseline (speedup 1.0000x reference)
"""Trainium2 Bass kernel for nn_Attention_51883204936008.

Multi-head attention (B=64, L=256, E=1024, H=16, HD=64) with interleaved
RoPE, causal + padding mask, softmax, and in/out projections.

Strategy: data-parallel over batch across 8 NeuronCores (8 batches/core).
Per core, a batch-blocked pipeline in transposed-activation layout:
  - x.T tiles [ei, tok] (host-pretransposed), weights W.T [ei, eo]
  - Q.T/K.T = W.T-tile-stationary matmuls (fp32r fast path); RoPE applied
    via DVE stream-shuffle (even/odd partition swap) + elementwise mults
  - scores S.T[tokk, tq] = K.T-slice-stationary matmuls; causal masking via
    triangular multiply on the diagonal blocks, skipping fully-masked
    blocks; padding mask folded into the exp bias (per-partition)
  - softmax without max-subtraction (scores are O(1) by construction);
    row sums come from an appended ones-column in V
  - O.T = V-stationary matmul with P.T moving; normalize via per-head DVE
    reciprocal + DMA partition-broadcast + tensor-tensor multiply
  - Z = O.T-stationary matmul with Wo.T moving -> natural-layout output
"""

import sys

sys.path.insert(0, "/opt/trn_rl_repo")

from contextlib import ExitStack

import numpy as np

import concourse.bacc as bacc
import concourse.tile as tile
from concourse import mybir
from concourse.bass_utils import run_bass_kernel_spmd

F32 = mybir.dt.float32
F32R = mybir.dt.float32r
AF = mybir.ActivationFunctionType

N_CORES = 8
B, L, E, H = 64, 256, 1024, 16
HD = E // H  # 64
BPC = B // N_CORES  # batches per core = 8
TOK = BPC * L  # tokens per core = 2048
NKT = E // 128  # 8 contraction tiles
ROPE_BASE = 10000.0
SCALE = 1.0 / float(np.sqrt(HD))
NEG = -30000.0

_program_cache = {}


def _swap_mask():
    m = []
    for j in range(16):
        m.extend([2 * j + 1, 2 * j])
    return m


def build_program(has_bq, has_bk, has_bv, has_bo, dbg=False):
    key = (has_bq, has_bk, has_bv, has_bo, dbg)
    if key in _program_cache:
        return _program_cache[key]

    nc = bacc.Bacc("TRN2", target_bir_lowering=False, debug=False, num_devices=N_CORES)

    xT = nc.dram_tensor("xT", [NKT, 128, TOK], F32R, kind="ExternalInput").ap()
    wq = nc.dram_tensor("wq", [NKT, 128, E], F32R, kind="ExternalInput").ap()
    wk = nc.dram_tensor("wk", [NKT, 128, E], F32R, kind="ExternalInput").ap()
    wv = nc.dram_tensor("wv", [NKT, 128, E], F32R, kind="ExternalInput").ap()
    wo = nc.dram_tensor("wo", [NKT, 128, E], F32R, kind="ExternalInput").ap()
    cosT_d = nc.dram_tensor("cosT", [128, L], F32, kind="ExternalInput").ap()
    sinS_d = nc.dram_tensor("sinS", [128, L], F32, kind="ExternalInput").ap()
    tri_d = nc.dram_tensor("tri", [128, 128], F32R, kind="ExternalInput").ap()
    ebias_d = nc.dram_tensor("ebias", [128, 2 * BPC], F32, kind="ExternalInput").ap()
    z_d = nc.dram_tensor("z", [BPC, L, E], F32, kind="ExternalOutput").ap()
    if has_bq:
        bq_d = nc.dram_tensor("bqt", [128, NKT], F32, kind="ExternalInput").ap()
    if has_bk:
        bk_d = nc.dram_tensor("bkt", [128, NKT], F32, kind="ExternalInput").ap()
    if has_bv:
        bv_d = nc.dram_tensor("bvr", [128, E], F32, kind="ExternalInput").ap()
    if has_bo:
        bo_d = nc.dram_tensor("bor", [128, E], F32, kind="ExternalInput").ap()
    if dbg:
        dqt = nc.dram_tensor("dqt", [128, TOK], F32, kind="ExternalOutput").ap()
        dkt = nc.dram_tensor("dkt", [128, TOK], F32, kind="ExternalOutput").ap()
        dva = nc.dram_tensor("dva", [2, 128, 16 * 65], F32, kind="ExternalOutput").ap()
        dp = nc.dram_tensor("dp", [2, 128, 256], F32, kind="ExternalOutput").ap()
        dot = nc.dram_tensor("dot", [128, TOK], F32, kind="ExternalOutput").ap()

    SWAP = _swap_mask()

    with tile.TileContext(nc) as tc, ExitStack() as ctx:
        # ---------------- pools ----------------
        consts = ctx.enter_context(tc.tile_pool(name="consts", bufs=1))
        wpool = ctx.enter_context(tc.tile_pool(name="wpool", bufs=1))
        xpool = ctx.enter_context(tc.tile_pool(name="xpool", bufs=1))
        shp = ctx.enter_context(tc.tile_pool(name="shp", bufs=1))
        qkp = ctx.enter_context(tc.tile_pool(name="qkp", bufs=1))
        vap = ctx.enter_context(tc.tile_pool(name="vap", bufs=1))
        pp = ctx.enter_context(tc.tile_pool(name="pp", bufs=2))
        otp = ctx.enter_context(tc.tile_pool(name="otp", bufs=1))
        zp = ctx.enter_context(tc.tile_pool(name="zp", bufs=2))
        smallp = ctx.enter_context(tc.tile_pool(name="smallp", bufs=2))
        ps_qk = ctx.enter_context(tc.tile_pool(name="ps_qk", bufs=2, space="PSUM"))
        ps_s = ctx.enter_context(tc.tile_pool(name="ps_s", bufs=2, space="PSUM"))
        ps_pv = ctx.enter_context(tc.tile_pool(name="ps_pv", bufs=2, space="PSUM"))
        ps_big = ctx.enter_context(tc.tile_pool(name="ps_big", bufs=2, space="PSUM"))

        # ---------------- constants / weights ----------------
        w_sb = {}
        for name, src in (("wq", wq), ("wk", wk), ("wv", wv), ("wo", wo)):
            tiles = []
            for kt in range(NKT):
                t = wpool.tile([128, E], F32R, tag=f"{name}{kt}")
                nc.sync.dma_start(t[:], src[kt])
                tiles.append(t)
            w_sb[name] = tiles

        cosT = consts.tile([128, L], F32, tag="cosT")
        nc.sync.dma_start(cosT[:], cosT_d[:])
        sinS = consts.tile([128, L], F32, tag="sinS")
        nc.sync.dma_start(sinS[:], sinS_d[:])
        tri = consts.tile([128, 128], F32R, tag="tri")
        nc.sync.dma_start(tri[:], tri_d[:])
        ebias = consts.tile([128, 2 * BPC], F32, tag="ebias")
        nc.sync.dma_start(ebias[:], ebias_d[:])
        onesc = consts.tile([128, 16], F32, tag="onesc")
        nc.vector.memset(onesc[:], 1.0)
        if has_bq:
            bqt = consts.tile([128, NKT], F32, tag="bqt")
            nc.sync.dma_start(bqt[:], bq_d[:])
        if has_bk:
            bkt = consts.tile([128, NKT], F32, tag="bkt")
            nc.sync.dma_start(bkt[:], bk_d[:])
        if has_bv:
            bvr = consts.tile([128, E], F32, tag="bvr")
            nc.sync.dma_start(bvr[:], bv_d[:])
        if has_bo:
            bor = consts.tile([128, E], F32, tag="bor")
            nc.sync.dma_start(bor[:], bo_d[:])

        # ---------------- per-batch pipeline ----------------
        for b in range(BPC):
            c0 = b * L  # token column offset of this batch in [*, TOK] layouts

            # -- load x.T slice for this batch: 8 tiles [128, 256]
            xb = []
            for kt in range(NKT):
                t = xpool.tile([128, L], F32R, tag=f"xb{kt}")
                nc.sync.dma_start(t[:], xT[kt][:, c0 : c0 + L])
                xb.append(t)

            # -- Q.T / K.T projections + RoPE
            qt_kt = {}
            for tname, wname, hasb in (("q", "wq", has_bq), ("k", "wk", has_bk)):
                qt = qkp.tile([128, TOK], F32R, tag=f"{tname}t")
                raw = qt[:]
                for eo2 in range(4):  # one psum tile holds 2 eo-tiles of 256
                    psq = ps_qk.tile([128, 512], F32, tag="psqk")
                    for half in range(2):
                        eo = eo2 * 2 + half
                        for kt in range(NKT):
                            nc.tensor.matmul(
                                psq[:, half * L : half * L + L],
                                w_sb[wname][kt][:, eo * 128 : (eo + 1) * 128],
                                xb[kt][:],
                                start=(kt == 0),
                                stop=(kt == NKT - 1),
                            )
                    if hasb:
                        bt = bqt if tname == "q" else bkt
                        for half in range(2):
                            eo = eo2 * 2 + half
                            nc.scalar.activation(
                                raw[:, eo * L : (eo + 1) * L],
                                psq[:, half * L : half * L + L],
                                AF.Identity,
                                bias=bt[:, eo : eo + 1],
                            )
                    else:
                        nc.scalar.activation(
                            raw[:, eo2 * 512 : (eo2 + 1) * 512], psq[:], AF.Copy
                        )
                # rope (in place): qt = raw*cosT + swap(raw)*sinS
                sh = shp.tile([128, TOK], F32, tag="sh")
                nc.vector.stream_shuffle(sh[:], raw.bitcast(F32), SWAP)
                for blk in range(BPC):
                    sl = slice(blk * L, (blk + 1) * L)
                    nc.gpsimd.tensor_mul(sh[:, sl], sh[:, sl], sinS[:])
                    nc.gpsimd.tensor_mul(raw[:, sl], raw[:, sl], cosT[:])
                nc.vector.tensor_add(qt[:], raw, sh[:])
                qt_kt[tname] = qt

            qt, kt_ = qt_kt["q"], qt_kt["k"]
            if dbg and b == 0:
                nc.sync.dma_start(dqt[:], qt[:].bitcast(F32))
                nc.sync.dma_start(dkt[:], kt_[:].bitcast(F32))

            # -- V projection into V_aug (ones column appended per head)
            va = [
                vap.tile([128, 16 * 65], F32R, tag=f"va{mt}", name=f"va{mt}")
                for mt in range(2)
            ]
            for mt in range(2):
                for en in range(2):
                    psv = ps_big.tile([128, 512], F32, tag="psbig")
                    for kt in range(NKT):
                        nc.tensor.matmul(
                            psv[:],
                            xb[kt][:, mt * 128 : (mt + 1) * 128],
                            w_sb["wv"][kt][:, en * 512 : (en + 1) * 512],
                            start=(kt == 0),
                            stop=(kt == NKT - 1),
                        )
                    dst = va[mt][:].rearrange("p (h c) -> p h c", c=65)[
                        :, en * 8 : (en + 1) * 8, 0:64
                    ]
                    if has_bv:
                        tmp = zp.tile([128, 512], F32, tag="vtmp")
                        nc.vector.tensor_add(
                            tmp[:], psv[:], bvr[:, en * 512 : (en + 1) * 512]
                        )
                        nc.vector.tensor_copy(
                            dst, tmp[:].rearrange("p (h c) -> p h c", c=64)
                        )
                    else:
                        nc.vector.tensor_copy(
                            dst, psv[:].rearrange("p (h c) -> p h c", c=64)
                        )
                # ones column at col 64 of each 65-block
                nc.vector.tensor_copy(
                    va[mt][:].rearrange("p (h c) -> p h c", c=65)[:, :, 64:65],
                    onesc[:].rearrange("p (h c) -> p h c", c=1),
                )
            if dbg and b == 0:
                for mt in range(2):
                    nc.sync.dma_start(dva[mt], va[mt][:].bitcast(F32))

            # -- attention per head
            ot = otp.tile([128, TOK], F32R, tag="ot")
            for h in range(H):
                prow = (h % 2) * 64
                ccol = (h // 2) * L
                q_sl = qt[prow : prow + 64, ccol : ccol + L]
                p_tiles = []
                for kt2 in range(2):
                    k_sl = kt_[
                        prow : prow + 64, ccol + kt2 * 128 : ccol + kt2 * 128 + 128
                    ]
                    pss = ps_s.tile([128, 256], F32, tag="pss")
                    nc.tensor.matmul(pss[:], k_sl, q_sl, start=True, stop=True)
                    pt = pp.tile([128, 256], F32R, tag=f"pt{kt2}")
                    eb = ebias[:, 2 * b + kt2 : 2 * b + kt2 + 1]
                    if kt2 == 0:
                        nc.scalar.activation(
                            pt[:], pss[:], AF.Exp, bias=eb, scale=SCALE
                        )
                        nc.vector.tensor_mul(pt[:, 0:128], pt[:, 0:128], tri[:])
                    else:
                        nc.scalar.activation(
                            pt[:, 128:256],
                            pss[:, 128:256],
                            AF.Exp,
                            bias=eb,
                            scale=SCALE,
                        )
                        nc.vector.tensor_mul(
                            pt[:, 128:256], pt[:, 128:256], tri[:]
                        )
                    p_tiles.append(pt)
                if dbg and b == 0 and h == 0:
                    nc.sync.dma_start(dp[0], p_tiles[0][:].bitcast(F32))
                    nc.sync.dma_start(
                        dp[1][:, 128:256], p_tiles[1][:, 128:256].bitcast(F32)
                    )

                psp = ps_pv.tile([65, 256], F32, tag="pspv")
                nc.tensor.matmul(
                    psp[:],
                    va[0][:, 65 * h : 65 * h + 65],
                    p_tiles[0][:],
                    start=True,
                    stop=False,
                    skip_group_check=True,
                )
                nc.tensor.matmul(
                    psp[:, 128:256],
                    va[1][:, 65 * h : 65 * h + 65],
                    p_tiles[1][:, 128:256],
                    start=False,
                    stop=True,
                    skip_group_check=True,
                )
                # normalize: reciprocal of the rowsum row, broadcast, multiply
                rec = smallp.tile([65, 256], F32, tag="rec")
                nc.vector.reciprocal(rec[64:65, :], psp[64:65, :])
                rep = smallp.tile([64, 256], F32, tag="rep")
                nc.gpsimd.dma_start(
                    rep[:],
                    rec[64:65, :].unsqueeze(1).broadcast_to([1, 64, 256]),
                )
                nc.vector.tensor_mul(
                    ot[prow : prow + 64, ccol : ccol + L], psp[0:64, :], rep[:]
                )
            if dbg and b == 0:
                nc.sync.dma_start(dot[:], ot[:].bitcast(F32))

            # -- output projection Z = O.T-stationary @ Wo.T
            for mt in range(2):
                for en in range(2):
                    psz = ps_big.tile([128, 512], F32, tag="psbig")
                    for j in range(NKT):
                        nc.tensor.matmul(
                            psz[:],
                            ot[:, j * L + mt * 128 : j * L + mt * 128 + 128],
                            w_sb["wo"][j][:, en * 512 : (en + 1) * 512],
                            start=(j == 0),
                            stop=(j == NKT - 1),
                        )
                    zt = zp.tile([128, 512], F32, tag="zt")
                    if has_bo:
                        nc.vector.tensor_add(
                            zt[:], psz[:], bor[:, en * 512 : (en + 1) * 512]
                        )
                    else:
                        nc.vector.tensor_copy(zt[:], psz[:])
                    nc.sync.dma_start(
                        z_d[b][mt * 128 : (mt + 1) * 128, en * 512 : (en + 1) * 512],
                        zt[:],
                    )

    nc.finalize()
    _program_cache[key] = nc
    return nc


def _host_prep(x, Wq, bq, Wk, bk, Wv, bv, Wo, bo, attention_mask):
    """Build per-core input maps (all numpy, fp32)."""
    x = np.asarray(x, np.float32)
    attention_mask = np.asarray(attention_mask)

    def wtiles(W):
        # W.T -> [ei, eo] -> kt-tiled [NKT, 128, E]
        WT = np.ascontiguousarray(np.asarray(W, np.float32).T)
        return np.ascontiguousarray(WT.reshape(NKT, 128, E))

    wqt, wkt, wvt, wot = wtiles(Wq), wtiles(Wk), wtiles(Wv), wtiles(Wo)

    # rope tables for the [128, TOK] head-pair layout
    d = np.arange(128)
    j = (d % 64) // 2
    inv = 1.0 / (ROPE_BASE ** (np.arange(0, HD, 2, dtype=np.float64) / HD))
    t = np.arange(L, dtype=np.float64)
    ang = t[None, :] * inv[j][:, None]  # [128, L]
    cos1 = np.cos(ang)
    sin1 = np.sin(ang)
    sgn = np.where(d % 2 == 0, -1.0, 1.0)[:, None]
    cosT = np.ascontiguousarray(cos1.astype(np.float32))
    sinS = np.ascontiguousarray((sin1 * sgn).astype(np.float32))

    r = np.arange(128)
    tri = (r[None, :] >= r[:, None]).astype(np.float32)  # allowed: tq >= tokk

    bq = np.asarray(bq, np.float32)
    bk = np.asarray(bk, np.float32)
    bv = np.asarray(bv, np.float32)
    bo = np.asarray(bo, np.float32)
    has_bq = bool(np.any(bq))
    has_bk = bool(np.any(bk))
    has_bv = bool(np.any(bv))
    has_bo = bool(np.any(bo))

    in_maps = []
    for c in range(N_CORES):
        xc = x[c * BPC : (c + 1) * BPC]  # [8, 256, 1024]
        xTc = np.ascontiguousarray(xc.reshape(TOK, E).T)  # [1024, 2048]
        xTt = np.ascontiguousarray(xTc.reshape(NKT, 128, TOK))
        amc = attention_mask[c * BPC : (c + 1) * BPC]  # [8, 256] bool
        eb = np.zeros((128, 2 * BPC), np.float32)
        for b in range(BPC):
            for kt2 in range(2):
                keymask = amc[b, kt2 * 128 : (kt2 + 1) * 128]
                eb[:, 2 * b + kt2] = np.where(keymask, 0.0, NEG)
        m = {
            "xT": xTt,
            "wq": wqt,
            "wk": wkt,
            "wv": wvt,
            "wo": wot,
            "cosT": cosT,
            "sinS": sinS,
            "tri": tri,
            "ebias": eb,
        }
        if has_bq:
            m["bqt"] = np.ascontiguousarray(bq.reshape(NKT, 128).T)
        if has_bk:
            m["bkt"] = np.ascontiguousarray(bk.reshape(NKT, 128).T)
        if has_bv:
            m["bvr"] = np.ascontiguousarray(np.tile(bv[None, :], (128, 1)))
        if has_bo:
            m["bor"] = np.ascontiguousarray(np.tile(bo[None, :], (128, 1)))
        in_maps.append(m)
    return in_maps, (has_bq, has_bk, has_bv, has_bo)


def kernel(x, Wq, bq, Wk, bk, Wv, bv, Wo, bo, attention_mask, _dbg=False, _trace=False):
    in_maps, flags = _host_prep(x, Wq, bq, Wk, bk, Wv, bv, Wo, bo, attention_mask)
    nc = build_program(*flags, dbg=_dbg)
    res = run_bass_kernel_spmd(nc, in_maps, core_ids=list(range(N_CORES)), trace=_trace)
    z = np.concatenate([res.results[c]["z"] for c in range(N_CORES)], axis=0)
    out = z.reshape(B, L, E).astype(np.float32)
    if _dbg:
        return out, res
    return out


if __name__ == "__main__":
    rng = np.random.default_rng(0)
    x = rng.standard_normal((B, L, E)).astype(np.float32)
    W = [rng.standard_normal((E, E)).astype(np.float32) * 0.02 for _ in range(4)]
    bz = np.zeros(E, np.float32)
    am = np.ones((B, L), bool)
    out = kernel(x, W[0], bz, W[1], bz, W[2], bz, W[3], bz, am)
    print("out", out.shape, out.dtype, float(np.abs(out).max()))
